# revision 12
# baseline (speedup 1.0000x reference)
"""C2Q attention kernel for Trainium2 (8 NeuronCores, SPMD over batch).

Computes, for inputs similarity [B=32, C=2048, Q=512] f32 and
qencode [B=32, Q=512, H=1024] f32:

    attn = softmax(similarity, axis=-1)
    out  = einsum('bcq,bqh->bch', attn, qencode)

Sharding: data-parallel over batch, 4 batches per core, no collectives.

Per-core pipeline, per 128-row tile of C:
  DMA sim tile [128c, 512q] -> PE transpose to [128q, 512c] (PSUM)
  -> ACT exp (PSUM->SBUF) -> PE matmuls: denominator via ones column +
  main contraction over q (fp32r single-pass PE mode by default)
  -> normalize fused into the PSUM->SBUF copies (ACT & DVE)
  -> DMA out [128c, 1024h].
"""

import numpy as np
from contextlib import ExitStack

import concourse.bass as bass
import concourse.tile as tile
from concourse import bacc, mybir
from concourse.bass_utils import run_bass_kernel_spmd
from concourse.masks import make_identity

B, C, Q, H = 32, 2048, 512, 1024
N_CORES = 8
BPC = B // N_CORES          # batches per core
P = 128                     # partitions
CT = C // P                 # c-tiles per batch
KQ = Q // P                 # q chunks (contraction tiles)
NH = H // 512               # h psum banks per c-tile

F32 = mybir.dt.float32

# Matmul input mode: "f32r" (single-pass fp32, 4x faster PE, ~bf16-precision
# mantissa), "f32" (exact two-pass fp32), or "bf16".
MM_MODE = "f32r"


def build_nc(mm_mode=MM_MODE, n_repeats=1):
    # dtype of the matmul operands (exp'd similarity, qencode, ones). FP32r
    # must be rounded at the producing instruction, so the producer tiles are
    # declared float32r directly.
    if mm_mode == "bf16":
        mm_dt = mybir.dt.bfloat16
    elif mm_mode == "f32r":
        mm_dt = mybir.dt.float32r
    else:
        mm_dt = F32

    nc = bacc.Bacc(None, target_bir_lowering=False)
    sim = nc.dram_tensor("similarity", [BPC, C, Q], F32, kind="ExternalInput")
    qe = nc.dram_tensor("qencode", [BPC, Q, H], F32, kind="ExternalInput")
    out = nc.dram_tensor("out", [BPC, C, H], F32, kind="ExternalOutput")

    with ExitStack() as ctx:
        tc = ctx.enter_context(tile.TileContext(nc))

        const_pool = ctx.enter_context(tc.tile_pool(name="const", bufs=1))
        ident = const_pool.tile([P, P], F32)
        make_identity(nc, ident[:])
        # memset cannot write float32r (invalid ISA); produce the ones column
        # by memsetting f32 and casting through an ACT copy when needed.
        ones = const_pool.tile([P, 2], mm_dt)
        if mm_dt == mybir.dt.float32r:
            ones_f32 = const_pool.tile([P, 2], F32)
            nc.vector.memset(ones_f32[:], 1.0)
            nc.scalar.activation(ones[:], ones_f32[:],
                                 mybir.ActivationFunctionType.Copy)
        else:
            nc.vector.memset(ones[:], 1.0)

        qe_pool = ctx.enter_context(tc.tile_pool(name="qe", bufs=2))
        sim_pool = ctx.enter_context(tc.tile_pool(name="simt", bufs=4))
        expT_pool = ctx.enter_context(tc.tile_pool(name="expT", bufs=3))
        out_pool = ctx.enter_context(tc.tile_pool(name="outsb", bufs=3))
        recip_pool = ctx.enter_context(tc.tile_pool(name="recip", bufs=4))
        tr_pool = ctx.enter_context(tc.tile_pool(name="trps", bufs=2, space="PSUM"))
        mm_pool = ctx.enter_context(tc.tile_pool(name="mmps", bufs=4, space="PSUM"))
        den_pool = ctx.enter_context(tc.tile_pool(name="denps", bufs=2, space="PSUM"))

        qe_tiles = {}

        def stage_load(b, ct):
            """DMA in + transpose + exp for tile (b, ct). Returns WORK state."""
            if ct == 0:
                qe_t = qe_pool.tile([P, KQ * H], mm_dt, name="qe_t")
                # gpsimd (SWDGE) casts f32 -> mm_dt during the DMA when needed
                qe_dma = nc.sync if mm_dt == F32 else nc.gpsimd
                for k in range(KQ):
                    qe_dma.dma_start(
                        qe_t[:, k * H:(k + 1) * H],
                        qe[b, k * P:(k + 1) * P, :],
                    )
                qe_tiles[b] = qe_t

            sim_t = sim_pool.tile([P, Q], F32, name="sim_t")
            nc.sync.dma_start(sim_t[:], sim[b, ct * P:(ct + 1) * P, :])

            tr = tr_pool.tile([P, Q], F32, name="tr")
            for k in range(KQ):
                nc.tensor.transpose(
                    tr[:, k * P:(k + 1) * P], sim_t[:, k * P:(k + 1) * P], ident[:]
                )

            expT = expT_pool.tile([P, Q], mm_dt, name="expT")
            nc.scalar.activation(expT[:], tr[:], mybir.ActivationFunctionType.Exp)
            return (b, ct, expT, qe_tiles[b])

        def stage_work(state):
            b, ct, expT, qe_t = state

            # Denominator: sum_q exp = expT.T @ ones  -> [128c, 2]
            den = den_pool.tile([P, 2], F32, name="den")
            for k in range(KQ):
                nc.tensor.matmul(
                    den[:],
                    expT[:, k * P:(k + 1) * P],
                    ones[:],
                    start=(k == 0),
                    stop=(k == KQ - 1),
                )
            recip = recip_pool.tile([P, 1], F32, name="recip")
            nc.vector.reciprocal(recip[:], den[:, 0:1])

            mm_ps = []
            for h in range(NH):
                ps = mm_pool.tile([P, 512], F32, name="mm_ps")
                for k in range(KQ):
                    nc.tensor.matmul(
                        ps[:],
                        expT[:, k * P:(k + 1) * P],
                        qe_t[:, k * H + h * 512: k * H + h * 512 + 512],
                        start=(k == 0),
                        stop=(k == KQ - 1),
                    )
                mm_ps.append(ps)

            out_sb = out_pool.tile([P, H], F32, name="out_sb")
            # Normalize during the PSUM->SBUF copy; split across ACT and DVE.
            nc.scalar.activation(
                out_sb[:, 0:512], mm_ps[0][:],
                mybir.ActivationFunctionType.Copy, scale=recip[:],
            )
            nc.vector.tensor_scalar_mul(out_sb[:, 512:1024], mm_ps[1][:], recip[:])
            nc.sync.dma_start(out[b, ct * P:(ct + 1) * P, :], out_sb[:])

        pending = None
        for _rep in range(n_repeats):
            for b in range(BPC):
                for ct in range(CT):
                    state = stage_load(b, ct)
                    if pending is not None:
                        stage_work(pending)
                    pending = state
        stage_work(pending)

    nc.finalize()
    return nc


_NC_CACHE = {}


def _get_nc(mode=MM_MODE):
    if mode not in _NC_CACHE:
        _NC_CACHE[mode] = build_nc(mode)
    return _NC_CACHE[mode]


def run(similarity, qencode, mode=MM_MODE, **spmd_kwargs):
    nc = _get_nc(mode)
    similarity = np.ascontiguousarray(similarity, dtype=np.float32)
    qencode = np.ascontiguousarray(qencode, dtype=np.float32)
    in_maps = [
        {
            "similarity": similarity[i * BPC:(i + 1) * BPC],
            "qencode": qencode[i * BPC:(i + 1) * BPC],
        }
        for i in range(N_CORES)
    ]
    res = run_bass_kernel_spmd(nc, in_maps, core_ids=list(range(N_CORES)), **spmd_kwargs)
    out = np.concatenate([res.results[i]["out"] for i in range(N_CORES)], axis=0)
    return out.astype(np.float32, copy=False), res


def kernel(similarity, qencode):
    out, _ = run(similarity, qencode)
    return out


# revision 27
# speedup vs baseline: 53.3367x; 53.3367x over previous
"""C2Q attention kernel for Trainium2 (8 NeuronCores, SPMD over batch).

Computes, for inputs similarity [B=32, C=2048, Q=512] f32 and
qencode [B=32, Q=512, H=1024] f32:

    attn = softmax(similarity, axis=-1)
    out  = einsum('bcq,bqh->bch', attn, qencode)

Sharding: data-parallel over batch, 4 batches per core, no collectives.

Per-core pipeline, per group of 4 C-tiles (128 rows each):
  1 MiB batched DMA in -> ACT exp (f32 -> mm dtype) with the softmax
  denominator accumulated for free via accum_out -> PE transpose of the
  exp'd tile to [q, c] layout -> PE matmul contraction over q
  (fp16 operands by default: exp(sim) in [5e-3, 230] and qencode in
  [-6, 6] are comfortably inside fp16 range, so precision ~2^-11 while
  the PE runs at full 1 cycle/row with overlapped weight loads)
  -> normalization fused into the PSUM->SBUF copies (ACT & DVE)
  -> 2 MiB batched DMA out. Software-pipelined three deep.
"""

import numpy as np
from contextlib import ExitStack

import concourse.bass as bass
import concourse.tile as tile
from concourse import bacc, mybir
from concourse.bass_utils import run_bass_kernel_spmd
from concourse.masks import make_identity

B, C, Q, H = 32, 2048, 512, 1024
N_CORES = 8
BPC = B // N_CORES          # batches per core
P = 128                     # partitions
CT = C // P                 # c-tiles per batch
KQ = Q // P                 # q chunks (contraction tiles)
NH = H // 512               # h psum banks per c-tile
GW = 4                      # c-tiles per DMA group (1 MiB loads / 2 MiB stores)
NG = BPC * CT // GW         # total groups per core

F32 = mybir.dt.float32

# Matmul operand dtype: "fp16" (default; ~5e-4 rel err), "f32r" (single-pass
# fp32 PE mode, ~1.5e-4, slower: its 4-byte weight load is fused into each
# matmul and serializes), "bf16" (~3e-3), or "f32" (exact, 4x slower PE).
MM_MODE = "fp16"


def build_nc(mm_mode=MM_MODE, n_repeats=1, loop_repeats=None):
    mm_dt = {
        "fp16": mybir.dt.float16,
        "bf16": mybir.dt.bfloat16,
        "f32r": mybir.dt.float32r,
        "f32": F32,
    }[mm_mode]

    nc = bacc.Bacc(None, target_bir_lowering=False)
    sim = nc.dram_tensor("similarity", [BPC, C, Q], F32, kind="ExternalInput")
    qe = nc.dram_tensor("qencode", [BPC, Q, H], F32, kind="ExternalInput")
    out = nc.dram_tensor("out", [BPC, C, H], F32, kind="ExternalOutput")

    with ExitStack() as ctx:
        tc = ctx.enter_context(tile.TileContext(nc))

        const_pool = ctx.enter_context(tc.tile_pool(name="const", bufs=1))
        ident_dt = F32 if mm_dt == mybir.dt.float32r else mm_dt
        ident = const_pool.tile([P, P], ident_dt)
        make_identity(nc, ident[:])

        qe_pool = ctx.enter_context(
            tc.tile_pool(name="qe", bufs=BPC if loop_repeats is not None else 2))
        sim_pool = ctx.enter_context(tc.tile_pool(name="simt", bufs=3))
        expn_pool = ctx.enter_context(tc.tile_pool(name="expn", bufs=GW + 2))
        expT_pool = ctx.enter_context(tc.tile_pool(name="expT", bufs=2 * GW + 2))
        out_pool = ctx.enter_context(tc.tile_pool(name="outsb", bufs=2))
        den_pool = ctx.enter_context(tc.tile_pool(name="den", bufs=3))
        recip_pool = ctx.enter_context(tc.tile_pool(name="recip", bufs=3))
        tr_pool = ctx.enter_context(tc.tile_pool(name="trps", bufs=3, space="PSUM"))
        mm_pool = ctx.enter_context(tc.tile_pool(name="mmps", bufs=4, space="PSUM"))

        qe_tiles = {}

        def load_qe(b):
            qe_t = qe_pool.tile([P, KQ * H], mm_dt, name="qe_t")
            # gpsimd (SWDGE) casts f32 -> mm_dt during the DMA when needed;
            # one transfer per batch.
            qe_dma = nc.sync if mm_dt == F32 else nc.gpsimd
            qe_dma.dma_start(
                qe_t[:].rearrange("p (k h) -> p k h", h=H),
                qe[b].rearrange("(k p) h -> p k h", p=P),
            )
            qe_tiles[b] = qe_t

        def stage_dma(b, g):
            """Batched 1 MiB load of GW c-tiles (natural [c, q] layout)."""
            if g == 0 and b not in qe_tiles:
                load_qe(b)
            sim_t = sim_pool.tile([P, GW * Q], F32, name="sim_t")
            nc.sync.dma_start(
                sim_t[:].rearrange("p (gg q) -> p gg q", q=Q),
                sim[b, g * GW * P:(g + 1) * GW * P, :].rearrange(
                    "(gg p) q -> p gg q", p=P),
            )
            return (b, g, sim_t)

        def stage_exp(st):
            """exp on ACT (f32 -> mm_dt) with the softmax denominator
            accumulated on the side; one reciprocal per group on DVE."""
            b, g, sim_t = st
            den = den_pool.tile([P, GW], F32, name="den")
            exps = []
            for t in range(GW):
                e = expn_pool.tile([P, Q], mm_dt, name="expn")
                nc.scalar.activation(
                    e[:], sim_t[:, t * Q:(t + 1) * Q],
                    mybir.ActivationFunctionType.Exp,
                    accum_out=den[:, t:t + 1],
                )
                exps.append(e)
            recip = recip_pool.tile([P, GW], F32, name="recip")
            nc.vector.reciprocal(recip[:], den[:])
            return (b, g, exps, recip)

        # float32r cannot be an identity/transpose operand; its bits are plain
        # f32 (pre-rounded by the ACT producer), so transpose under an f32
        # view and re-tag on the PSUM->SBUF copy.
        tr_dt = F32 if mm_dt == mybir.dt.float32r else mm_dt

        def stage_tr(st):
            """PE transpose of the exp'd tiles into [q, c] layout + DVE
            copies PSUM -> SBUF (matmul weights must live in SBUF)."""
            b, g, exps, recip = st
            expTs = []
            for t in range(GW):
                tr = tr_pool.tile([P, Q], tr_dt, name="tr")
                src = exps[t]
                src_ap = src[:].bitcast(F32) if tr_dt != mm_dt else src[:]
                for k in range(KQ):
                    nc.tensor.transpose(
                        tr[:, k * P:(k + 1) * P],
                        src_ap[:, k * P:(k + 1) * P],
                        ident[:],
                    )
                expT = expT_pool.tile([P, Q], mm_dt, name="expT")
                nc.vector.tensor_copy(expT[:], tr[:])
                expTs.append(expT)
            return (b, g, expTs, recip, qe_tiles[b])

        def stage_work(st):
            """Contraction over q on PE, normalization fused into the
            PSUM->SBUF copies, batched 2 MiB store."""
            b, g, expTs, recip, qe_t = st
            out_sb = out_pool.tile([P, GW * H], F32, name="out_sb")
            for t in range(GW):
                expT = expTs[t]
                r = recip[:, t:t + 1]
                for h in range(NH):
                    ps = mm_pool.tile([P, 512], F32, name="mm_ps")
                    for k in range(KQ):
                        nc.tensor.matmul(
                            ps[:],
                            expT[:, k * P:(k + 1) * P],
                            qe_t[:, k * H + h * 512: k * H + h * 512 + 512],
                            start=(k == 0),
                            stop=(k == KQ - 1),
                        )
                    o = t * H + h * 512
                    if h == 0:
                        nc.scalar.activation(
                            out_sb[:, o:o + 512], ps[:],
                            mybir.ActivationFunctionType.Copy, scale=r,
                        )
                    else:
                        nc.vector.tensor_scalar_mul(out_sb[:, o:o + 512], ps[:], r)
            nc.scalar.dma_start(
                out[b, g * GW * P:(g + 1) * GW * P, :].rearrange(
                    "(gg p) h -> p gg h", p=P),
                out_sb[:].rearrange("p (gg h) -> p gg h", h=H),
            )

        def one_rep(keep_qe=False):
            # 3-deep software pipeline over groups:
            #   iteration i emits DMA(i), EXP(i-1), TR(i-1), WORK(i-2)
            # so no engine's in-order stream head-of-line blocks on a DMA.
            bg = [(b, g) for b in range(BPC) for g in range(CT // GW)]
            st_dma = st_exp = st_tr = None
            for i in range(len(bg) + 2):
                new_dma = stage_dma(*bg[i]) if i < len(bg) else None
                if st_dma is not None:
                    new_exp = stage_exp(st_dma)
                else:
                    new_exp = None
                if new_exp is not None:
                    new_tr = stage_tr(new_exp)
                else:
                    new_tr = None
                if st_tr is not None:
                    stage_work(st_tr)
                st_dma, st_tr = new_dma, new_tr
            if not keep_qe:
                qe_tiles.clear()

        if loop_repeats is not None:
            # Benchmark-only: run the whole per-core program loop_repeats
            # times in one dispatch (dynamic loop). NOTE: SWDGE (gpsimd)
            # DMA inside For_i crashes the device, so qe is preloaded.
            for b in range(BPC):
                load_qe(b)
            with tc.For_i(0, loop_repeats, 1):
                one_rep(keep_qe=True)
        else:
            for _rep in range(n_repeats):
                one_rep()

    nc.finalize()
    return nc


_NC_CACHE = {}


def _get_nc(mode=MM_MODE):
    if mode not in _NC_CACHE:
        _NC_CACHE[mode] = build_nc(mode)
    return _NC_CACHE[mode]


def run(similarity, qencode, mode=MM_MODE, **spmd_kwargs):
    nc = _get_nc(mode)
    similarity = np.ascontiguousarray(similarity, dtype=np.float32)
    qencode = np.ascontiguousarray(qencode, dtype=np.float32)
    in_maps = [
        {
            "similarity": similarity[i * BPC:(i + 1) * BPC],
            "qencode": qencode[i * BPC:(i + 1) * BPC],
        }
        for i in range(N_CORES)
    ]
    res = run_bass_kernel_spmd(nc, in_maps, core_ids=list(range(N_CORES)), **spmd_kwargs)
    out = np.concatenate([res.results[i]["out"] for i in range(N_CORES)], axis=0)
    return out.astype(np.float32, copy=False), res


def kernel(similarity, qencode):
    out, _ = run(similarity, qencode)
    return out
